# revision 1
# baseline (speedup 1.0000x reference)
"""Trainium2 Bass kernel for nn_AttnLayer_60636348285537.

Computes o = einsum('nt,bcthw->bcn', f, video) / (W*H) with the gaussian
attention filters f derived from mu_t/sigma_t, returning [B, C*N].

Sharding: pure data parallel over batch — B=8 batches on 8 NeuronCores,
one batch per core. Each core reduces its [C=1024, T*W*H=6272] slab:
  stage 1 (DVE): vs[c, t]  = sum_wh video[c, t, wh]      (free-dim reduce)
  stage 2 (DVE): out[c, n] = sum_t  vs[c, t] * fs[n, t]  (fs = f/196)
The tiny filter tensor fs is computed on host and replicated to all cores.
"""

import os
import sys

for _p in ("/opt/trn_rl_repo", "/root/.axon_site/_ro/trn_rl_repo"):
    if os.path.isdir(_p):
        sys.path.insert(0, _p)
        break

import numpy as np

P = 128          # SBUF partitions
C = 1024         # channels
T = 32           # time
WH = 196         # W*H = 14*14
X = T * WH       # free elems per channel
N = 3            # gaussian filters
N_CT = C // P    # channel tiles per core
N_CORES = 8

_cache = {}


def _build_module(vid_bufs=4, dma="gpsimd", splits=1, repeats=1,
                  incr_stage2=False, alt_engines=False, s2_chunk=None,
                  tail_splits=None):
    """splits: sub-DMAs per 128-channel tile (must divide T).
    tail_splits: finer split count for the last channel tile (shrinks the
    post-DMA-chain critical path); implies its own stage2 chunk."""
    import concourse.bacc as bacc
    import concourse.mybir as mybir
    from concourse import tile

    f32 = mybir.dt.float32
    nc = bacc.Bacc("TRN2", target_bir_lowering=False, debug=False,
                   num_devices=N_CORES)
    vid = nc.dram_tensor("video", [C, X], f32, kind="ExternalInput").ap()
    fw = nc.dram_tensor("fw", [P, N * T], f32, kind="ExternalInput").ap()
    out = nc.dram_tensor("out", [C, N], f32, kind="ExternalOutput").ap()

    dma_eng = {"gpsimd": nc.gpsimd, "sync": nc.sync, "scalar": nc.scalar}[dma]
    engines = ([nc.sync, nc.scalar] if alt_engines else [dma_eng])
    assert T % splits == 0
    if tail_splits:
        assert T % tail_splits == 0

    with tile.TileContext(nc) as tc:
        with (
            tc.tile_pool(name="vid", bufs=vid_bufs) as vid_pool,
            tc.tile_pool(name="persist", bufs=1) as persist,
            tc.tile_pool(name="tmp", bufs=2) as tmp_pool,
        ):
            f_sb = persist.tile([P, N * T], f32, tag="f_sb")
            f_view = f_sb.rearrange("p (n t) -> p n t", n=N)

            vid_ct = vid.rearrange("(ct p) x -> ct p x", p=P)
            first = True
            gi = 0
            for _rep in range(repeats):
                vs_all = persist.tile([P, N_CT * T], f32, tag="vs_all")
                out_sb = persist.tile([P, N_CT * N], f32, tag="out_sb")
                vs_view = vs_all.rearrange("p (ct t) -> p ct t", t=T)
                out_view = out_sb.rearrange("p (ct n) -> p ct n", n=N)

                def stage2(ct_list, fused=False):
                    # out[c, n] = sum_t vs[c, ct, t] * fs[n, t]
                    nct = len(ct_list)
                    ct0 = ct_list[0]
                    if fused and nct == 1:
                        # single fused mul+reduce per filter (3 DVE ops)
                        for n in range(N):
                            scr = tmp_pool.tile([P, T], f32, tag="scr")
                            nc.vector.tensor_tensor_reduce(
                                out=scr[:],
                                in0=vs_view[:, ct0, :],
                                in1=f_view[:, n, :],
                                scale=1.0,
                                scalar=0.0,
                                op0=mybir.AluOpType.mult,
                                op1=mybir.AluOpType.add,
                                accum_out=out_view[:, ct0, n].unsqueeze(-1),
                            )
                        return
                    prod = tmp_pool.tile([P, nct * T], f32, tag="prod")
                    prod_view = prod.rearrange("p (ct t) -> p ct t", t=T)
                    for n in range(N):
                        f_b = f_view[:, n, :].unsqueeze(1).broadcast_to(
                            [P, nct, T])
                        nc.vector.tensor_mul(
                            prod_view[:], vs_view[:, ct0:ct0 + nct, :], f_b)
                        nc.vector.reduce_sum(
                            out_view[:, ct0:ct0 + nct, n], prod_view[:],
                            axis=mybir.AxisListType.X,
                        )

                pending = []
                for ct in range(N_CT):
                    last_ct = ct == N_CT - 1
                    tail_ct = bool(tail_splits) and last_ct
                    n_s = tail_splits if tail_ct else splits
                    ts = T // n_s
                    xs = X // n_s
                    ct_view = vid_ct[ct].rearrange("p (s x) -> s p x", s=n_s)
                    if tail_ct:
                        prod7 = tmp_pool.tile([P, N * T], f32, tag="prod7")
                        p7_view = prod7.rearrange("p (n t) -> p n t", n=N)
                    for s in range(n_s):
                        vt = vid_pool.tile([P, X // splits], f32, tag="vt")
                        engines[gi % len(engines)].dma_start(
                            vt[:, :xs], ct_view[s])
                        gi += 1
                        if first:
                            # load the tiny filter tile after the first
                            # video DMA is in flight
                            dma_eng.dma_start(f_sb[:], fw[:])
                            first = False
                        o = ct * T + s * ts
                        nc.vector.reduce_sum(
                            vs_all[:, o:o + ts],
                            vt[:, :xs].rearrange("p (q w) -> p q w", w=WH),
                            axis=mybir.AxisListType.X,
                        )
                        if tail_ct:
                            # pre-multiply this slice by all filters now so
                            # only one tiny reduce remains after the chain
                            t0 = s * ts
                            nc.vector.tensor_mul(
                                p7_view[:, :, t0:t0 + ts],
                                vs_view[:, ct, t0:t0 + ts].unsqueeze(1)
                                .broadcast_to([P, N, ts]),
                                f_view[:, :, t0:t0 + ts],
                            )
                    if tail_ct:
                        nc.vector.reduce_sum(
                            out_view[:, ct, :], p7_view[:],
                            axis=mybir.AxisListType.X,
                        )
                        pending = []
                        continue
                    pending.append(ct)
                    flush = (
                        (incr_stage2 and True)
                        or (s2_chunk and len(pending) == s2_chunk)
                        or last_ct
                        or (tail_splits and ct == N_CT - 2)
                    )
                    if flush and (incr_stage2 or s2_chunk or last_ct):
                        # contiguous runs only (stage2 slices ct ranges)
                        stage2(pending)
                        pending = []

                dma_eng.dma_start(
                    out.rearrange("(ct p) n -> p ct n", p=P), out_view[:]
                )
    nc.compile()
    return nc


BEST = dict(vid_bufs=12, dma="sync", splits=4, s2_chunk=2, tail_splits=8)


def _get_module():
    if "nc" not in _cache:
        _cache["nc"] = _build_module(**BEST)
    return _cache["nc"]


def _filters_scaled(mu_t: np.ndarray, sigma_t: np.ndarray) -> np.ndarray:
    """f / (W*H) as [N, T] float32, matching the reference filter math."""
    mu = np.tanh(mu_t.astype(np.float64))
    sg = 1.0 / (1.0 + np.exp(-sigma_t.astype(np.float64)))
    sigma = np.exp(1.5 - 2.0 * sg)
    centers = (T - 1) * (mu + 1.0) / 2.0
    t = np.arange(T, dtype=np.float64)[None, :] - centers[:, None]
    f = np.exp(-(t**2) / (2.0 * sigma[:, None] ** 2 + 1e-16))
    f = f / (np.sum(f, axis=1, keepdims=True) + 1e-16)
    return (f / WH).astype(np.float32)


def kernel(video: np.ndarray, mu_t: np.ndarray, sigma_t: np.ndarray,
           meta: np.ndarray) -> np.ndarray:
    from concourse import bass_utils

    B = video.shape[0]
    assert B == N_CORES, f"kernel hardcodes one batch per core, got B={B}"
    fs = _filters_scaled(np.asarray(mu_t), np.asarray(sigma_t))
    fw = np.tile(fs.reshape(1, N * T), (P, 1))
    vid = np.ascontiguousarray(np.asarray(video), dtype=np.float32)
    vid = vid.reshape(B, C, X)

    nc = _get_module()
    in_maps = [{"video": vid[b], "fw": fw} for b in range(B)]
    res = bass_utils.run_bass_kernel_spmd(nc, in_maps,
                                          core_ids=list(range(N_CORES)))
    out = np.stack([res.results[b]["out"].reshape(C * N) for b in range(B)])
    return out.astype(np.float32)



# revision 2
# speedup vs baseline: 1.6112x; 1.6112x over previous
"""Trainium2 Bass kernel for nn_AttnLayer_60636348285537.

Computes o = einsum('nt,bcthw->bcn', f, video) / (W*H) with the gaussian
attention filters f derived from mu_t/sigma_t, returning [B, C*N].

Sharding: pure data parallel over batch — B=8 batches on 8 NeuronCores,
one batch per core. Each core reduces its [C=1024, T*W*H=6272] slab.

Pipeline (per core):
  - gpsimd (SWDGE) casting DMAs stream the f32 video into bf16 SBUF tiles
    (the DMA-engine cost is charged on the bf16 output bytes, halving the
    stream time vs an f32 copy; bf16 keeps rel err ~1e-3 << 2e-2 tol).
  - DVE stage 1 per chunk: pairwise fold adds (bf16 tensor_tensor runs in
    2x perf mode) 196 -> 98 -> 49 -> 24(+1) -> 12, then a 1x reduce of the
    12-wide groups plus the leftover column: vs[c,t] = sum_wh v[c,t,wh].
  - DVE stage 2 per chunk: prod[c,n,t] = vs[c,t] * fs[n,t] (fs = f/196,
    f32), then one reduce per ct: out[c,n] = sum_t prod.
  - Stream order: ct7 bulk (t<28) first, ct0..ct6, then tiny ct7 tail
    slices (2t/1t/1t) so almost no work is serialized after the last DMA.
  - Output: single [128, 24] f32 DMA (channel-block-major; host transposes).
"""

import os
import sys

for _p in ("/opt/trn_rl_repo", "/root/.axon_site/_ro/trn_rl_repo"):
    if os.path.isdir(_p):
        sys.path.insert(0, _p)
        break

import numpy as np

P = 128          # SBUF partitions
C = 1024         # channels
T = 32           # time
WH = 196         # W*H = 14*14
X = T * WH       # free elems per channel
N = 3            # gaussian filters
N_CT = C // P    # channel tiles per core
N_CORES = 8

_cache = {}


def _build_module(vid_bufs=4, tail_ts=(2, 1, 1), bulk_first=True,
                  use_fold34=True):
    """bf16-stream module. tail_ts: sizes (in timesteps) of the trailing
    ct7 slices; the rest of ct7 is its leading bulk chunk."""
    import concourse.bacc as bacc
    import concourse.mybir as mybir
    from concourse import tile

    f32 = mybir.dt.float32
    bf16 = mybir.dt.bfloat16
    XL = mybir.AxisListType.X
    nc = bacc.Bacc("TRN2", target_bir_lowering=False, debug=False,
                   num_devices=N_CORES)
    vid = nc.dram_tensor("video", [C, X], f32, kind="ExternalInput").ap()
    fw = nc.dram_tensor("fw", [P, N * T], f32, kind="ExternalInput").ap()
    out = nc.dram_tensor("out", [P, N_CT * N], f32, kind="ExternalOutput").ap()

    tail_total = sum(tail_ts)
    bulk_t = T - tail_total
    # (ct, t0, nt) chunk plan in stream order
    plan = []
    if bulk_first:
        plan.append((N_CT - 1, 0, bulk_t))
    for ct in range(N_CT - 1):
        plan.append((ct, 0, T))
    if not bulk_first:
        plan.append((N_CT - 1, 0, bulk_t))
    t0 = bulk_t
    for nt in tail_ts:
        plan.append((N_CT - 1, t0, nt))
        t0 += nt

    vid_ct = vid.rearrange("(ct p) x -> ct p x", p=P)

    with nc.allow_low_precision(reason="bf16 pipeline, rel tol 2e-2"):
        with tile.TileContext(nc) as tc:
            with (
                tc.tile_pool(name="vid", bufs=vid_bufs) as vid_pool,
                tc.tile_pool(name="fold", bufs=2) as fold_pool,
                tc.tile_pool(name="persist", bufs=1) as persist,
                tc.tile_pool(name="tmp", bufs=2) as tmp_pool,
            ):
                f_sb = persist.tile([P, N * T], f32, tag="f_sb")
                f_view = f_sb.rearrange("p (n t) -> p n t", n=N)
                vs_all = persist.tile([P, T], f32, tag="vs_all")
                out_sb = persist.tile([P, N_CT * N], f32, tag="out_sb")
                out_view = out_sb.rearrange("p (ct n) -> p ct n", n=N)
                prod7 = persist.tile([P, N * T], f32, tag="prod7")
                p7_view = prod7.rearrange("p (n t) -> p n t", n=N)

                first = True
                for ct, t0, nt in plan:
                    ne = nt * WH
                    last_ct = ct == N_CT - 1
                    vt = vid_pool.tile([P, X], bf16, tag="vt")
                    nc.gpsimd.dma_start(
                        vt[:, :ne], vid_ct[ct][:, t0 * WH:t0 * WH + ne])
                    if first:
                        # fw rides the otherwise-idle DMA window during the
                        # first SWDGE descriptor generation
                        nc.sync.dma_start(f_sb[:], fw[:])
                        first = False

                    # --- stage 1: fold chain + reduce -> vs[:, t0:t0+nt] ---
                    v3 = vt[:, :ne].rearrange(
                        "p (t two w) -> p t two w", two=2, w=98)
                    h = fold_pool.tile([P, T * 98], bf16, tag="h")
                    h_used = h[:, :nt * 98]
                    nc.vector.tensor_add(
                        h_used.rearrange("p (t w) -> p t w", w=98),
                        v3[:, :, 0, :], v3[:, :, 1, :])
                    h3 = h_used.rearrange(
                        "p (t two w) -> p t two w", two=2, w=49)
                    q = fold_pool.tile([P, T * 49], bf16, tag="q")
                    q_used = q[:, :nt * 49]
                    q_view = q_used.rearrange("p (t w) -> p t w", w=49)
                    nc.vector.tensor_add(q_view, h3[:, :, 0, :], h3[:, :, 1, :])

                    vs_dst = vs_all[:, t0:t0 + nt]
                    if use_fold34 and nt >= 8:
                        r = fold_pool.tile([P, T * 24], bf16, tag="r")
                        r_used = r[:, :nt * 24]
                        r_view = r_used.rearrange("p (t w) -> p t w", w=24)
                        nc.vector.tensor_add(
                            r_view, q_view[:, :, 0:24], q_view[:, :, 24:48])
                        s = fold_pool.tile([P, T * 12], bf16, tag="s")
                        s_used = s[:, :nt * 12]
                        nc.vector.tensor_add(
                            s_used.rearrange("p (t w) -> p t w", w=12),
                            r_view[:, :, 0:12], r_view[:, :, 12:24])
                        vs0 = tmp_pool.tile([P, T], bf16, tag="vs0")
                        nc.vector.reduce_sum(
                            vs0[:, :nt],
                            s_used.rearrange("p (t w) -> p t w", w=12),
                            axis=XL)
                        # add the odd 49th column q[:, t, 48]
                        nc.vector.tensor_add(
                            vs_dst, vs0[:, :nt], q_view[:, :, 48])
                    else:
                        nc.vector.reduce_sum(vs_dst, q_view, axis=XL)

                    # --- stage 2: premultiply by fs, then per-ct reduce ---
                    if last_ct:
                        nc.vector.tensor_mul(
                            p7_view[:, :, t0:t0 + nt],
                            vs_dst.unsqueeze(1).broadcast_to([P, N, nt]),
                            f_view[:, :, t0:t0 + nt])
                        if t0 + nt == T:
                            nc.vector.reduce_sum(
                                out_view[:, ct, :], p7_view[:], axis=XL)
                    else:
                        prod = tmp_pool.tile([P, N * T], f32, tag="prod")
                        pv = prod.rearrange("p (n t) -> p n t", n=N)
                        nc.vector.tensor_mul(
                            pv[:],
                            vs_all[:].unsqueeze(1).broadcast_to([P, N, T]),
                            f_view[:])
                        nc.vector.reduce_sum(out_view[:, ct, :], pv[:],
                                             axis=XL)

                nc.sync.dma_start(out, out_sb[:])
    nc.compile()
    return nc


BEST = dict(vid_bufs=4, tail_ts=(2, 1, 1), bulk_first=True, use_fold34=True)


def _get_module():
    if "nc" not in _cache:
        _cache["nc"] = _build_module(**BEST)
    return _cache["nc"]


def _filters_scaled(mu_t: np.ndarray, sigma_t: np.ndarray) -> np.ndarray:
    """f / (W*H) as [N, T] float32, matching the reference filter math."""
    mu = np.tanh(mu_t.astype(np.float64))
    sg = 1.0 / (1.0 + np.exp(-sigma_t.astype(np.float64)))
    sigma = np.exp(1.5 - 2.0 * sg)
    centers = (T - 1) * (mu + 1.0) / 2.0
    t = np.arange(T, dtype=np.float64)[None, :] - centers[:, None]
    f = np.exp(-(t**2) / (2.0 * sigma[:, None] ** 2 + 1e-16))
    f = f / (np.sum(f, axis=1, keepdims=True) + 1e-16)
    return (f / WH).astype(np.float32)


def kernel(video: np.ndarray, mu_t: np.ndarray, sigma_t: np.ndarray,
           meta: np.ndarray) -> np.ndarray:
    from concourse import bass_utils

    B = video.shape[0]
    assert B == N_CORES, f"kernel hardcodes one batch per core, got B={B}"
    fs = _filters_scaled(np.asarray(mu_t), np.asarray(sigma_t))
    fw = np.tile(fs.reshape(1, N * T), (P, 1))
    vid = np.ascontiguousarray(np.asarray(video), dtype=np.float32)
    vid = vid.reshape(B, C, X)

    nc = _get_module()
    in_maps = [{"video": vid[b], "fw": fw} for b in range(B)]
    res = bass_utils.run_bass_kernel_spmd(nc, in_maps,
                                          core_ids=list(range(N_CORES)))
    # out[p, ct*3+n] holds channel c = ct*128 + p
    outs = []
    for b in range(B):
        a = np.asarray(res.results[b]["out"]).reshape(P, N_CT, N)
        outs.append(a.transpose(1, 0, 2).reshape(C * N))
    return np.stack(outs).astype(np.float32)


# revision 7
# speedup vs baseline: 1.6172x; 1.0037x over previous
"""Trainium2 Bass kernel for nn_AttnLayer_60636348285537.

Computes o = einsum('nt,bcthw->bcn', f, video) / (W*H) with the gaussian
attention filters f derived from mu_t/sigma_t, returning [B, C*N].

Sharding: pure data parallel over batch — B=8 batches on 8 NeuronCores,
one batch per core. Each core reduces its [C=1024, T*W*H=6272] slab.

Per-core pipeline:
  - gpsimd (SWDGE) casting DMAs stream the f32 video into bf16 SBUF tiles;
    the DMA-engine hold is charged on the bf16 output bytes, halving the
    stream vs an f32 copy (bf16 keeps rel err ~5e-3 << 2e-2 tol).
  - DVE stage 1 per chunk: pairwise fold adds (bf16 tensor_tensor runs in
    the 2x DVE perf mode) 196 -> 98 -> 49 -> 24 -> 12, a 1x reduce of the
    12-wide groups, plus the odd column 48: vs[c,t] = sum_wh v[c,t,wh].
  - The Activation engine owns ct0 and ct1's first half via per-timestep
    activation+accum ops (and the last two 1-timestep taper slices),
    freeing DVE headroom so it tracks the stream.
  - DVE stage 2: prod[c,n,t] = vs[c,t]*fs[n,t] (f32), one reduce per ct.
  - Stream order: ct7 bulk first, ct0 (Act) quarters, ct1..ct5 halves,
    ct6 in 8t granules, then a ct7 taper (4t/2t/1t/1t) so little work
    serializes after the last byte lands.
  - Output: SWDGE scatter-add on its own queue, prepared mid-stream and
    triggered after the last reduce (skips HWDGE+DGE latency in the
    tail). The out region is zeroed by an early DMA since PJRT output
    buffers are not reliably zero-initialized.
"""

import os
import sys

for _p in ("/opt/trn_rl_repo", "/root/.axon_site/_ro/trn_rl_repo"):
    if os.path.isdir(_p):
        sys.path.insert(0, _p)
        break

import numpy as np

P = 128          # SBUF partitions
C = 1024         # channels
T = 32           # time
WH = 196         # W*H = 14*14
X = T * WH       # free elems per channel
N = 3            # gaussian filters
N_CT = C // P    # channel tiles per core
N_CORES = 8
OUT_W = 64       # scatter-add row width (256B alignment); first 24 used

_cache = {}


def _build_module(vid_bufs=11, out_mode="dma", act_halves=3,
                  act_tail=2, tail_ts=(4, 2, 1, 1), ct6_grans=4):
    """act_halves: number of 16t half-ct granules owned by the Act engine
    (ct0 counts as two). act_tail: trailing 1t taper slices owned by Act."""
    import concourse.bacc as bacc
    import concourse.mybir as mybir
    from concourse import tile

    f32 = mybir.dt.float32
    bf16 = mybir.dt.bfloat16
    i16 = mybir.dt.int16
    XL = mybir.AxisListType.X
    COPY = mybir.ActivationFunctionType.Copy

    nc = bacc.Bacc("TRN2", target_bir_lowering=False, debug=False,
                   num_devices=N_CORES, num_swdge_queues=2)
    vid = nc.dram_tensor("video", [C, X], f32, kind="ExternalInput").ap()
    fw = nc.dram_tensor("fw", [P, N * T], f32, kind="ExternalInput").ap()
    if out_mode == "scatter":
        sidx = nc.dram_tensor("sidx", [16, 8], i16, kind="ExternalInput").ap()
        out = nc.dram_tensor("out", [P, OUT_W], f32,
                             kind="ExternalOutput").ap()
    else:
        out = nc.dram_tensor("out", [P, N_CT * N], f32,
                             kind="ExternalOutput").ap()

    vid_ct = vid.rearrange("(ct p) x -> ct p x", p=P)
    tail_ct = N_CT - 1
    bulk_t = T - sum(tail_ts)
    n_act_tail = min(act_tail, sum(1 for nt in tail_ts if nt == 1))

    # (ct, t0, nt, owner) in stream order
    plan = [(tail_ct, 0, 16, "dve"), (tail_ct, 16, bulk_t - 16, "dve")]
    plan += [(0, 8 * qt, 8, "act") for qt in range(4)]
    plan += [(1, 0, 16, "act" if act_halves >= 3 else "dve"),
             (1, 16, 16, "dve")]
    ct6 = N_CT - 2
    for ct in range(2, ct6):
        plan += [(ct, 0, 16, "dve"), (ct, 16, 16, "dve")]
    g = T // ct6_grans
    plan += [(ct6, g * i, g, "dve") for i in range(ct6_grans)]
    t0 = bulk_t
    n_ones = 0
    for nt in tail_ts:
        ones_left = sum(1 for x in tail_ts if x == 1) - n_ones
        owner = "act" if (nt == 1 and ones_left <= n_act_tail) else "dve"
        if nt == 1:
            n_ones += 1
        plan.append((tail_ct, t0, nt, owner))
        t0 += nt

    with nc.allow_low_precision(reason="bf16 pipeline, rel tol 2e-2"):
        with tile.TileContext(nc) as tc:
            with (
                tc.tile_pool(name="vid", bufs=vid_bufs) as vid_pool,
                tc.tile_pool(name="fold", bufs=2) as fold_pool,
                tc.tile_pool(name="persist", bufs=1) as persist,
                tc.tile_pool(name="tmp", bufs=2) as tmp_pool,
            ):
                f_sb = persist.tile([P, N * T], f32, tag="f_sb")
                f_view = f_sb.rearrange("p (n t) -> p n t", n=N)
                vs_all = persist.tile([P, N_CT * T], f32, tag="vs_all")
                vs_view = vs_all.rearrange("p (ct t) -> p ct t", t=T)
                prod7 = persist.tile([P, N * T], f32, tag="prod7")
                p7_view = prod7.rearrange("p (n t) -> p n t", n=N)
                if out_mode == "scatter":
                    out_sb = persist.tile([P, OUT_W], f32, tag="out_sb")
                    nc.gpsimd.memset(out_sb[:], 0.0)
                    idx_sb = persist.tile([16, 8], i16, tag="idx_sb")
                else:
                    out_sb = persist.tile([P, N_CT * N], f32, tag="out_sb")
                out_view = out_sb[:, :N_CT * N].rearrange(
                    "p (ct n) -> p ct n", n=N)
                scrap = persist.tile([P, WH], f32, tag="scrap")

                def stage1_dve(vt, ct, t0, nt):
                    """fold chain + reduce: vs[ct, t0:t0+nt] (f32)."""
                    ne = nt * WH
                    vs_dst = vs_view[:, ct, t0:t0 + nt]
                    if nt == 1:
                        nc.vector.reduce_sum(
                            vs_dst, vt[:, :ne].unsqueeze(1), axis=XL)
                        return
                    v3 = vt[:, :ne].rearrange(
                        "p (t two w) -> p t two w", two=2, w=98)
                    h = fold_pool.tile([P, 16 * 98], bf16, tag="h")
                    hu = h[:, :nt * 98]
                    nc.vector.tensor_add(
                        hu.rearrange("p (t w) -> p t w", w=98),
                        v3[:, :, 0, :], v3[:, :, 1, :])
                    h3 = hu.rearrange("p (t two w) -> p t two w", two=2, w=49)
                    q = fold_pool.tile([P, 16 * 49], bf16, tag="q")
                    qu = q[:, :nt * 49]
                    q_view = qu.rearrange("p (t w) -> p t w", w=49)
                    nc.vector.tensor_add(q_view, h3[:, :, 0, :],
                                         h3[:, :, 1, :])
                    if nt >= 8:
                        r = fold_pool.tile([P, 16 * 24], bf16, tag="r")
                        r_view = r[:, :nt * 24].rearrange(
                            "p (t w) -> p t w", w=24)
                        nc.vector.tensor_add(
                            r_view, q_view[:, :, 0:24], q_view[:, :, 24:48])
                        s = fold_pool.tile([P, 16 * 12], bf16, tag="s")
                        su = s[:, :nt * 12]
                        nc.vector.tensor_add(
                            su.rearrange("p (t w) -> p t w", w=12),
                            r_view[:, :, 0:12], r_view[:, :, 12:24])
                        vs0 = tmp_pool.tile([P, 16], bf16, tag="vs0")
                        nc.vector.reduce_sum(
                            vs0[:, :nt],
                            su.rearrange("p (t w) -> p t w", w=12), axis=XL)
                        nc.vector.tensor_add(vs_dst, vs0[:, :nt],
                                             q_view[:, :, 48])
                    else:
                        nc.vector.reduce_sum(vs_dst, q_view, axis=XL)

                def stage1_act(vt, ct, t0, nt):
                    for t in range(t0, t0 + nt):
                        nc.scalar.activation(
                            scrap[:], vt[:, (t - t0) * WH:(t - t0 + 1) * WH],
                            COPY, accum_out=vs_view[:, ct, t:t + 1])

                def premult_final(ct, pv=None, tslice=None, final=True):
                    if pv is None:
                        prod = tmp_pool.tile([P, N * T], f32, tag="prod")
                        pv = prod.rearrange("p (n t) -> p n t", n=N)
                    sl = slice(0, T) if tslice is None else tslice
                    nt = sl.stop - sl.start
                    nc.vector.tensor_mul(
                        pv[:, :, sl],
                        vs_view[:, ct, sl].unsqueeze(1).broadcast_to(
                            [P, N, nt]),
                        f_view[:, :, sl])
                    if final:
                        nc.vector.reduce_sum(out_view[:, ct, :], pv[:],
                                             axis=XL)

                first = True
                for ct, t0, nt, owner in plan:
                    ne = nt * WH
                    vt = vid_pool.tile([P, X], bf16, tag="vt")
                    nc.gpsimd.dma_start(
                        vt[:, :ne], vid_ct[ct][:, t0 * WH:t0 * WH + ne])
                    if first:
                        nc.sync.dma_start(f_sb[:], fw[:])
                        if out_mode == "scatter":
                            nc.sync.dma_start(idx_sb[:], sidx)
                            # PJRT outputs are not reliably zeroed; the
                            # scatter-add needs a clean base
                            nc.sync.dma_start(out, out_sb[:])
                        first = False

                    if owner == "act":
                        stage1_act(vt, ct, t0, nt)
                        continue
                    stage1_dve(vt, ct, t0, nt)

                    if ct == tail_ct:
                        if t0 + nt == bulk_t:
                            premult_final(ct, pv=p7_view,
                                          tslice=slice(0, bulk_t),
                                          final=False)
                    elif t0 + nt == T:
                        premult_final(ct)
                        if ct == 5:
                            # Act finished ct0 by now; late placement keeps
                            # the sem wait out of the stream-tracking path
                            premult_final(0)
                        if ct == ct6:
                            premult_final(1)
                            if out_mode == "scatter":
                                # descriptor prep on the idle end of Pool's
                                # gen queue, isolated on SWDGE queue 1
                                nc.gpsimd.dma_scatter_add(
                                    out.unsqueeze(1), out_sb[:].unsqueeze(1),
                                    idx_sb[:], P, P, OUT_W,
                                    prepare_only=True,
                                    sem=nc.alloc_semaphore("out_sdma"),
                                    queue_num=1)

                premult_final(tail_ct, pv=p7_view,
                              tslice=slice(bulk_t, T), final=False)
                nc.vector.reduce_sum(out_view[:, tail_ct, :], p7_view[:],
                                     axis=XL)

                if out_mode == "scatter":
                    nc.gpsimd.trigger_dma(count=None, queue_num=1)
                else:
                    nc.sync.dma_start(out, out_sb[:])
    nc.compile()
    return nc


BEST = dict(vid_bufs=11, out_mode="dma", act_halves=3, act_tail=2,
            tail_ts=(4, 2, 1, 1), ct6_grans=4)


def _get_module():
    if "nc" not in _cache:
        _cache["nc"] = _build_module(**BEST)
    return _cache["nc"]


def _filters_scaled(mu_t: np.ndarray, sigma_t: np.ndarray) -> np.ndarray:
    """f / (W*H) as [N, T] float32, matching the reference filter math."""
    mu = np.tanh(mu_t.astype(np.float64))
    sg = 1.0 / (1.0 + np.exp(-sigma_t.astype(np.float64)))
    sigma = np.exp(1.5 - 2.0 * sg)
    centers = (T - 1) * (mu + 1.0) / 2.0
    t = np.arange(T, dtype=np.float64)[None, :] - centers[:, None]
    f = np.exp(-(t**2) / (2.0 * sigma[:, None] ** 2 + 1e-16))
    f = f / (np.sum(f, axis=1, keepdims=True) + 1e-16)
    return (f / WH).astype(np.float32)


def kernel(video: np.ndarray, mu_t: np.ndarray, sigma_t: np.ndarray,
           meta: np.ndarray) -> np.ndarray:
    from concourse import bass_utils

    B = video.shape[0]
    assert B == N_CORES, f"kernel hardcodes one batch per core, got B={B}"
    fs = _filters_scaled(np.asarray(mu_t), np.asarray(sigma_t))
    fw = np.tile(fs.reshape(1, N * T), (P, 1))
    vid = np.ascontiguousarray(np.asarray(video), dtype=np.float32)
    vid = vid.reshape(B, C, X)

    nc = _get_module()
    in_maps = []
    for b in range(B):
        m = {"video": vid[b], "fw": fw}
        if BEST["out_mode"] == "scatter":
            sidx = np.zeros((16, 8), np.int16)
            for i in range(P):
                sidx[i % 16, i // 16] = i
            m["sidx"] = sidx
        in_maps.append(m)
    res = bass_utils.run_bass_kernel_spmd(nc, in_maps,
                                          core_ids=list(range(N_CORES)))
    # out[p, ct*3+n] holds channel c = ct*128 + p
    outs = []
    for b in range(B):
        a = np.asarray(res.results[b]["out"])[:, :N_CT * N]
        a = a.reshape(P, N_CT, N)
        outs.append(a.transpose(1, 0, 2).reshape(C * N))
    return np.stack(outs).astype(np.float32)
